# revision 32
# baseline (speedup 1.0000x reference)
"""BERT self-attention (B=4, S=2048, H=1024, 16 heads x 64) on 8 TRN2 NeuronCores.

Sharding: data-parallel over batch (4) x tensor-parallel over head-groups (2).
Core c handles batch c//2 and heads [8*(c%2), 8*(c%2)+8): it gets the full
hidden_states[b] plus the 512 W-columns/bias entries for its heads, and
produces out[b, :, 512*g : 512*(g+1)]. No cross-core communication.

Per-core kernel (bf16/fp16 matmuls, f32 accumulation in PSUM):
  xT   = transpose(x) via PE (bf16)               [1024h, 2048s]
  QT/KT = W.T @ xT  (+bias)                       [512hd, 2048s]
  V'   = xT.T @ Wv (+bias), 65 cols per head with an appended ones column
  per (head-pair, q-macro 512, k-chunk 128):
    scoresT[k, q] = KT_h[:, kc].T @ QT_h[:, qm]   (two heads row-packed, K=64)
    expT = exp(0.125 * scoresT)                   (ACT, fp16 out, N=1024/inst)
    ctxT[65, q] += V'_h[kc].T @ expT              (row 64 = softmax denominator)
    + one Q/K projection matmul of the NEXT head-pair (fills the PE gap
      while ACT paces the loop at ~1.1us/k-chunk)
  epilogue: ctxT -> hardware DMA-transpose -> [q, 65]; multiply by
  reciprocal denominator (GPSIMD); batched DMA out via HWDGE.

Schedule notes (from perfetto traces):
 - ACT exp (FD=1024 from PSUM) is the loop pacer at ~1.07-1.11us/iter;
   the PE instruction stream (ctx pair + row-packed score pair + one
   proj MM) is ~1.08us/iter, so both engines run ~100% in steady state.
   PSUM's 8 banks are exactly spent (2x2 score double-buffer + 2 ctx
   accumulators + 2 proj accumulators), which caps the exp FD at 1024.
 - The epilogue is split: the PSUM-draining copies + DMA-transposes are
   issued at the period boundary, but the reciprocal/scale/store half is
   deferred one full period so its DVE ops sit after the next period's
   proj bias-adds in the DVE FIFO and can never head-of-line-block them
   (that stall idled ACT ~1.7us per boundary and HAM-re-throttled PE).
 - The last ctx of a period can only run after the last exp, which
   starves the PE through the turnover (the 2-slot score ring can't run
   ahead). Chunks kc=1..4's proj+scores are priority-hoisted to the
   period start so the boundary window stays full; the ctx backlog
   drains through the 10-deep exp-tile ring. Hoisting more chunks
   back-fires (priority ties scramble the score-ring order).
 - All staging DMAs go through the sync-engine HWDGE rings (SWDGE
   DIRECT2D enqueues cost ~640ns each on the GPSIMD engine), with x
   prefetched ahead of the wq/wk staging (DMA rings round-robin, so
   enqueue order decides arrival order under bandwidth contention).
 - The Q/K projection interleave is paced globally at 1 matmul per
   iteration across all 16 periods; the queue also carries the qT
   projections of qm1-3 (phase1 only computes kT for windows 1-3 plus
   the qm0 qT), which shaved ~5us off the serial phase1 tail.  Per-iter
   PE (score pair 230ns + ctx pair 430ns + 1 proj MM 215ns + overheads
   ~1.03us) stays just under the exp pace (1.08us); all queue deadlines
   hold with >=4-iteration emission margin (k(3,3) lands at iter 192 vs
   first read at 204).
 - x is DMA'd as 1MB two-tile blocks (each dma_start costs ~0.6-1us of
   Sync-engine enqueue time; halving the enqueue count pulled the x
   arrivals ~4us earlier).
 - The last period's epilogue_b deferral is dropped (the proj queue is
   exhausted by then, so there is nothing to head-of-line-block),
   keeping the post-last-exp tail to ~9us.
Measured on TRN2: ~361.5us HW exec (v1 of this schedule: ~368us;
naive baseline 412us), rel l2 err ~3.7e-3.

Optimization attempts that did NOT beat this schedule (2026-08-12
session; all correct at 3.7e-3 but slower — details in case a future
session retries):
 - Streaming phase1 ("flash-style" period-0 crawl: x DMA'd in windows,
   transposes/V'/mt0-proj emitted per window interleaved with period 0's
   scores/exps; W staged as per-head-pair column slices; dummy exp at
   t=0 to preload the ACT table): 382.7us.  The exp stream starts at
   ~17us instead of ~90us, but the crawl's PE backlog (V' 27.5us +
   transposes + mt0 proj = ~50us of PE that all sits ahead of later
   scores in the PE FIFO) scatters ~60-90us of ACT idle through
   t=17-100us, and the steady state picks up ~0.03-0.1us/iter of ring
   coupling noise.  Engine totals measured: ACT-busy ~285us (256 exps x
   ~1.11us) vs PE-busy ~300us — the PE, not ACT, is the binding total,
   so any schedule floor is ~310us + head/tail; none of the streaming
   variants got close to that floor due to FIFO coupling losses.
 - Same + wavefront (qm1's exps pre-run in the crawl via an 18-deep exp
   ring, ctx replayed later): 407.7us (deferred-ctx ring pins + ACT
   FIFO ordering of PSUM->SBUF transpose-drains vs exps serialize the
   window chain).  With transpose-drains moved DVE->ACT: 471us.
 - v1 + mild overlap (period 0's 16 attn iters emitted into phase1's
   st>=13 tail, queue prefix pre-run to keep the mt3 kT deadline):
   375.7us; with ps_sc/ps_pj ring decoupling + casts off ACT: 378.1us.
   The phase1 st-loop is balanced enough that inserting exps stretches
   it by more than the head saving.  (Confirmed via PE-interval
   analysis: PE is ~92% occupied wall-to-wall — 324.6us busy in a
   353.6us span, only 6us of >1us gaps — so the 16-18us ACT-dead
   transition gap at the phase1/steady boundary is PE catch-up time,
   not reclaimable idle.)
 - matmul cannot write 16-bit PSUM (bass asserts fp32 out), so the
   exp FD cannot be doubled to 2048 by halving the score footprint.
 - Period-0 scores+exps moved into phase1's st12-15 era (ctx trailing
   one window, kproj/transposes on the ps_pj ring, casts on DVE, one
   queue group pre-run for the mt3 deadline): 366.5us vs this
   schedule's 361.5us.  The phase1->steady gap shrank (18.4->16.5us)
   but periods 1-2 picked up ~11us of new stalls — the PE work that
   feeds them did not shrink, it was only displaced later.  This closes
   the book on head-overlap schedules: the transition gap is PE-bound
   catch-up, full stop.
 - PE warm-up burst (100 identity matmuls at t~0 to trip the HAM
   clock gate before the first transposes): 430us — the dummies all
   write one PSUM tile, so they form a WAW chain that the scheduler
   serializes with per-dependency semaphores, stalling the whole PE
   stream.  Splitting the first 1MB x block into two 0.5MB tiles (to
   start the cast chain ~1.8us earlier): 364.7us — the extra enqueue
   and changed arrival pattern cost more than the earlier start.
 - Facts for future attempts: GPSIMD tensor_copy/CAST is ~5 cyc/elem
   ([128,1024] cast = 3.0us, bf16 copy [128,512] = 1.87us) — useless
   for bulk staging; DVE CAST [128,1024] = 0.68us; ACT copy =
   (N+352)/1.2ns; each nc.sync.dma_start costs ~0.6-1.2us of Sync-
   engine enqueue time and a [0,P]-broadcast DMA costs ~4.3us to
   enqueue; the runtime preamble before the first DMA can issue is
   ~7us; exp ACTIVATE measures ~1.11us (FD=1024 from PSUM); the
   row-packed K=64 score pair runs concurrently (~230ns/pair); ctx
   pair is serial (~430ns); per-iter steady PE ~0.86us + 7/8-paced
   proj ~0.19us, just under the exp pace.
"""

import sys
import types

sys.path.insert(0, "/opt/trn_rl_repo")

import numpy as np

import concourse.bass as bass
import concourse.tile as tile
from concourse import bacc, mybir
from concourse.bass_utils import run_bass_kernel_spmd
from concourse.masks import make_identity

B, S, H = 4, 2048, 1024
NH, HD = 16, 64
NCORES = 8
HEADS_PER_CORE = NH // 2      # 8 heads per core
HG = HEADS_PER_CORE * HD      # 512 = per-core head width
P = 128
QM = 512                      # q macro-tile
N_QM = S // QM                # 4
N_KC = S // P                 # 16 k chunks
N_ST = S // P                 # 16 s tiles
N_HB = H // P                 # 8 h chunks (contraction)
N_MT = HG // P                # 4 hd m-tiles

FP32 = mybir.dt.float32
BF16 = mybir.dt.bfloat16
FP16 = mybir.dt.float16


def _ensure_profile_hook():
    """The image's antenv lacks axon_hooks; shim it so trace=True works."""
    try:
        from antenv.axon_hooks import get_axon_ntff_profile_hook  # noqa: F401
        return
    except ImportError:
        pass
    try:
        from trn_agent_boot.trn_boot import _ntff_profile_via_ctypes
    except ImportError:
        return
    hook = _ntff_profile_via_ctypes("/opt/axon/libaxon_pjrt.so")
    mod = types.ModuleType("antenv.axon_hooks")
    mod.get_axon_ntff_profile_hook = lambda: hook
    mod.set_axon_ntff_profile_hook = lambda h: None
    sys.modules["antenv.axon_hooks"] = mod


def build():
    nc = bacc.Bacc("TRN2", target_bir_lowering=False, debug=False,
                   num_devices=NCORES)

    x_d = nc.declare_dram_parameter("x", [S, H], FP32, isOutput=False)
    wq_d = nc.declare_dram_parameter("wq", [H, HG], FP32, isOutput=False)
    wk_d = nc.declare_dram_parameter("wk", [H, HG], FP32, isOutput=False)
    wv_d = nc.declare_dram_parameter("wv", [H, HG], FP32, isOutput=False)
    bq_d = nc.declare_dram_parameter("bq", [HG], FP32, isOutput=False)
    bk_d = nc.declare_dram_parameter("bk", [HG], FP32, isOutput=False)
    bv_d = nc.declare_dram_parameter("bv", [HG], FP32, isOutput=False)
    out_d = nc.declare_dram_parameter("out", [S, HG], FP32, isOutput=True)

    with tile.TileContext(nc) as tc:
        _build_body(nc, tc, x_d, (wq_d, wk_d, wv_d), (bq_d, bk_d, bv_d), out_d)

    nc.finalize()
    return nc


def _build_body(nc, tc, x_d, w_d, b_d, out_d):
    wq_d, wk_d, wv_d = w_d
    bq_d, bk_d, bv_d = b_d

    import contextlib
    from contextlib import nullcontext as _nullcontext
    ctx = contextlib.ExitStack()
    with ctx:
        const = ctx.enter_context(tc.tile_pool(name="const", bufs=1))
        xf = ctx.enter_context(tc.tile_pool(name="xf", bufs=4))
        xbp = ctx.enter_context(tc.tile_pool(name="xbp", bufs=3))
        big = ctx.enter_context(tc.tile_pool(name="big", bufs=1))
        wstage = ctx.enter_context(tc.tile_pool(name="wstage", bufs=3))
        expp = ctx.enter_context(tc.tile_pool(name="expp", bufs=10))
        epil = ctx.enter_context(tc.tile_pool(name="epil", bufs=4))
        outp = ctx.enter_context(tc.tile_pool(name="outp", bufs=4))
        # PSUM budget (8 banks): ps_sc = 2 x 2-bank slots (scores double
        # buffer), ps_ctx = 2 x 1-bank slots (ctx accumulators), ps_pj =
        # 2 x 1-bank slots (V'/QK projection accumulators).
        ps_sc = ctx.enter_context(
            tc.tile_pool(name="ps_sc", bufs=2, space="PSUM"))
        ps_ctx = ctx.enter_context(
            tc.tile_pool(name="ps_ctx", bufs=2, space="PSUM"))
        ps_pj = ctx.enter_context(
            tc.tile_pool(name="ps_pj", bufs=2, space="PSUM"))

        # ---- x prefetch first so the first s-tiles land ASAP -----------
        x_blks = {}
        x_tiles = {}

        def fetch_x(st):
            # 1MB two-tile blocks: each dma_start costs ~0.6-1us of Sync
            # engine enqueue time, so halve the enqueue count.
            w = st // 2
            if w in x_blks:
                return
            xt = xf.tile([P, 2, H], FP32, tag="x", name=f"xblk{w}")
            nc.sync.dma_start(
                out=xt,
                in_=x_d.ap()[2 * w * P:2 * (w + 1) * P, :].rearrange(
                    "(a p) c -> p a c", p=P),
            )
            x_blks[w] = xt
            x_tiles[2 * w] = xt[:, 0, :]
            x_tiles[2 * w + 1] = xt[:, 1, :]

        for st in range(4):
            fetch_x(st)

        # ---- constants -------------------------------------------------
        ident_b128 = const.tile([P, P], BF16)
        make_identity(nc, ident_b128)
        ident_h = const.tile([HD + 1, HD + 1], FP16)
        make_identity(nc, ident_h)
        ident_b = ident_h
        bqT = const.tile([P, N_MT], FP32)
        nc.sync.dma_start(out=bqT, in_=bq_d.ap().rearrange("(o p) -> p o", p=P))
        bkT = const.tile([P, N_MT], FP32)
        nc.sync.dma_start(out=bkT, in_=bk_d.ap().rearrange("(o p) -> p o", p=P))
        bv_ap = bv_d.ap()
        bvb = const.tile([P, HG], FP32)
        nc.sync.dma_start(
            out=bvb,
            in_=bass.AP(tensor=bv_ap.tensor, offset=bv_ap.offset,
                        ap=[[0, P]] + [list(a) for a in bv_ap.ap]),
        )

        # ---- weights: staged via sync HWDGE, cast on DVE ---------------
        w_sb = {}
        for name, wd in (("q", wq_d), ("k", wk_d), ("v", wv_d)):
            w_sb[name] = big.tile([P, N_HB, HG], BF16, tag=f"w{name}",
                                  name=f"w{name}")

        def load_w(name, wd, ks):
            for k in ks:
                stg = wstage.tile([P, HG], FP32, tag="wstg", name=f"w{name}{k}")
                nc.sync.dma_start(out=stg, in_=wd.ap()[k * P:(k + 1) * P, :])
                nc.vector.tensor_copy(out=w_sb[name][:, k, :], in_=stg)

        load_w("v", wv_d, range(N_HB))

        # ---- per s-tile: load x, transpose to xT, project V' -----------
        xT = big.tile([P, N_HB, S], BF16, tag="xT")
        vp = big.tile([P, N_ST, HEADS_PER_CORE, HD + 1], BF16, tag="vp")
        # only the appended ones-column needs initializing; cols 0:HD are
        # fully written by the V' bias add below
        nc.vector.memset(vp[:, :, :, HD:HD + 1], 1.0)

        qT = big.tile([P, N_MT, S], BF16, tag="qT")
        kT = big.tile([P, N_MT, S], BF16, tag="kT")

        def proj_chunk(mt, n, pool=None, tag=None, names=("q", "k")):
            pool = pool or ps_pj
            tag = tag or "pj"
            for w_name, dst, bias in (("q", qT, bqT), ("k", kT, bkT)):
                if w_name not in names:
                    continue
                ps = pool.tile([P, QM], FP32, tag=tag,
                               name=f"proj{w_name}{mt}{n}")
                for k in range(N_HB):
                    nc.tensor.matmul(
                        ps,
                        lhsT=w_sb[w_name][:, k, mt * P:(mt + 1) * P],
                        rhs=xT[:, k, n * QM:(n + 1) * QM],
                        start=(k == 0),
                        stop=(k == N_HB - 1),
                    )
                nc.vector.tensor_scalar_add(
                    out=dst[:, mt, n * QM:(n + 1) * QM],
                    in0=ps,
                    scalar1=bias[:, mt:mt + 1],
                )

        def vprime(st):
            psv = ps_pj.tile([P, HG], FP32, tag="pj", name=f"v{st}")
            for hb in range(N_HB):
                nc.tensor.matmul(
                    psv,
                    lhsT=xT[:, hb, st * P:(st + 1) * P],
                    rhs=w_sb["v"][:, hb, :],
                    start=(hb == 0),
                    stop=(hb == N_HB - 1),
                )
            nc.vector.scalar_tensor_tensor(
                out=vp[:, st, :, 0:HD],
                in0=psv.rearrange("p (h d) -> p h d", h=HEADS_PER_CORE),
                scalar=1.0,
                in1=bvb.rearrange("p (h d) -> p h d", h=HEADS_PER_CORE),
                op0=mybir.AluOpType.mult,
                op1=mybir.AluOpType.add,
            )

        def phase1(ctx_ps0, pj0):
          # V' for tile st-1 is emitted after the transposes of tile st so
          # the PE never waits on the PSUM->SBUF eviction of its own tile.
          for st in range(N_ST):
            # stagger wq/wk staging (2 chunks per s-tile, after the early
            # x tiles + wv) so the x-tile prefetches keep their share of
            # HBM bandwidth
            if 3 <= st < 7:
                load_w("q", wq_d, (2 * (st - 3), 2 * (st - 3) + 1))
            elif 7 <= st < 11:
                load_w("k", wk_d, (2 * (st - 7), 2 * (st - 7) + 1))
            if st + 4 < N_ST:
                fetch_x(st + 4)
            xt = x_tiles[st]
            xb = xbp.tile([P, H], BF16, tag="xb", name=f"xb{st}")
            nc.scalar.copy(out=xb, in_=xt)
            for half in range(2):
                ps = ps_sc.tile([P, 4, P], BF16, tag="sc", name=f"xt{st}{half}")
                for q in range(4):
                    hb = half * 4 + q
                    nc.tensor.transpose(
                        ps[:, q, :], xb[:, hb * P:(hb + 1) * P], ident_b128)
                nc.vector.tensor_copy(
                    out=xT[:, half * 4:half * 4 + 4, st * P:(st + 1) * P],
                    in_=ps,
                )
            if st > 0:
                vprime(st - 1)
            if st >= 12:
                # kT windows are consumed inside period 0 itself; the qT
                # of qm1-3 ride the (now 8/8-paced) steady queue instead,
                # shaving ~5us of PE off the serial phase1 tail.
                proj_chunk(0, st - 12, pool=ps_ctx, tag="ctx",
                           names=("q", "k") if st == 12 else ("k",))
          vprime(N_ST - 1)

        # ---- attention for one head pair -------------------------------
        PD = 80  # 65 padded to a multiple of XBAR_TILE_SRC_ROWS (16)

        def new_ctx_ps(hp, qm):
            return [ps_ctx.tile([HD + 1, QM], FP32, tag="ctx",
                                name=f"ctx{hp}{qm}{hh}")
                    for hh in range(2)]

        # The Q/K projections for later head-pairs are paced globally at
        # ~7 matmuls per 8 iterations across ALL periods (instead of 1 per
        # iteration during hp0-2 and none during hp3): the exp instruction
        # only runs at its pure ~1005ns when the PE keeps the score feed a
        # full iteration ahead, and that needs per-iteration PE work at or
        # below the exp duration. Chunk deadlines (hp_next's qT/kT before
        # hp_next's periods) are met with large margin at this pacing.
        def make_proj_state():
            queue = [("q", 0, n) for n in (1, 2, 3)]
            for nxt in (1, 2, 3):
                queue.append(("q", nxt, 0))
                queue.extend(("k", nxt, n) for n in range(N_QM))
                queue.extend(("q", nxt, n) for n in (1, 2, 3))
            return {"queue": queue, "ci": 0, "mi": 0, "tile": None}

        def attn_kc(hp, qm, kc, ctx_ps, pstate):
            # The last ctx of a period can only run after the last exp, so
            # the PE would starve through the period turnover. Hoisting the
            # first few chunks' proj+scores to the period start keeps the
            # PE (and therefore the ACT's score feed) saturated while the
            # ctx backlog drains through the exp-tile ring.
            hoist = (tc.high_priority(offset=6 * kc) if kc in (1, 2, 3, 4)
                     else _nullcontext())
            with hoist:
                if pstate["ci"] < len(pstate["queue"]):
                    w_name, nxt, n = pstate["queue"][pstate["ci"]]
                    mi = pstate["mi"]
                    if mi == 0:
                        pstate["tile"] = ps_pj.tile(
                            [P, QM], FP32, tag="pj",
                            name=f"pj{w_name}{nxt}{n}")
                    nc.tensor.matmul(
                        pstate["tile"],
                        lhsT=w_sb[w_name][:, mi, nxt * P:(nxt + 1) * P],
                        rhs=xT[:, mi, n * QM:(n + 1) * QM],
                        start=(mi == 0),
                        stop=(mi == N_HB - 1),
                    )
                    if mi == N_HB - 1:
                        dst, bias = (qT, bqT) if w_name == "q" else (kT, bkT)
                        nc.vector.tensor_scalar_add(
                            out=dst[:, nxt, n * QM:(n + 1) * QM],
                            in0=pstate["tile"],
                            scalar1=bias[:, nxt:nxt + 1],
                        )
                        pstate["ci"] += 1
                        pstate["mi"] = 0
                    else:
                        pstate["mi"] = mi + 1
                sc = ps_sc.tile([P, 2, QM], FP32, tag="sc",
                                name=f"sc{hp}{qm}{kc}")
                for hh in range(2):
                    lo = hh * HD
                    nc.tensor.matmul(
                        sc[:, hh, :],
                        lhsT=kT[lo:lo + HD, hp, kc * P:(kc + 1) * P],
                        rhs=qT[lo:lo + HD, hp, qm * QM:(qm + 1) * QM],
                        start=True,
                        stop=True,
                        tile_position=(lo, 0),
                    )
            et = expp.tile([P, 2, QM], FP16, tag="exp")
            nc.scalar.activation(
                out=et, in_=sc,
                func=mybir.ActivationFunctionType.Exp,
                scale=0.125,
            )
            for hh in range(2):
                nc.tensor.matmul(
                    ctx_ps[hh],
                    lhsT=vp[:, kc, 2 * hp + hh, :],
                    rhs=et[:, hh, :],
                    start=(kc == 0),
                    stop=(kc == N_KC - 1),
                )

        def epilogue_a(ctx_ps, use_pe, ep_idx):
            """Drain the ctx accumulators out of PSUM and kick off the
            transposes. Emitted at the period boundary so the PSUM slots
            free quickly for the next period's accumulation."""
            tfulls = []
            for hh in range(2):
                csb = epil.tile([PD, QM], FP16, tag="ctxsb")
                if ep_idx < 2:
                    # rows 65:PD feed the xbar transpose as padding; each
                    # of the 4 ring buffers only needs zeroing once.
                    nc.vector.memset(csb[64:PD, :], 0.0)
                nc.vector.tensor_copy(out=csb[0:HD + 1, :], in_=ctx_ps[hh])
                if use_pe:
                    tp = ps_pj.tile([P, QM // P, HD + 2], FP16, tag="pj",
                                    name=f"tp{hh}")
                    for qs in range(QM // P):
                        nc.tensor.transpose(
                            tp[:, qs, 0:HD + 1],
                            csb[0:HD + 1, qs * P:(qs + 1) * P],
                            ident_b,
                        )
                    tfull = epil.tile([P, QM // P, HD + 1], FP16, tag="tpe")
                    nc.vector.tensor_copy(out=tfull, in_=tp[:, :, 0:HD + 1])
                else:
                    tfull = epil.tile([P, QM // P, PD], FP16, tag="tpsb")
                    for qs in range(QM // P):
                        nc.sync.dma_start_transpose(
                            out=tfull[:, qs, :],
                            in_=csb[:, qs * P:(qs + 1) * P],
                        )
                tfulls.append(tfull)
            return tfulls

        def epilogue_b(hp, qm, tfulls):
            """Reciprocal + scale + store. Deferred one period so these DVE
            ops are emitted after the next period's proj bias-adds and can
            never head-of-line-block them (the transposes they read were
            issued a full period earlier and are long done)."""
            for hh in range(2):
                tfull = tfulls[hh]
                rc = outp.tile([P, QM // P], FP32, tag="recip")
                nc.vector.reciprocal(out=rc, in_=tfull[:, :, HD:HD + 1])
                ot = outp.tile([P, QM // P, HD], FP32, tag="out")
                for qs in range(QM // P):
                    nc.vector.tensor_scalar_mul(
                        ot[:, qs, :], tfull[:, qs, 0:HD], rc[:, qs:qs + 1])
                row = qm * QM
                col = (2 * hp + hh) * HD
                nc.sync.dma_start(
                    out=out_d.ap()[row:row + QM, col:col + HD].rearrange(
                        "(a p) c -> p a c", p=P),
                    in_=ot,
                )

        phase1(None, None)
        pending = None
        pstate = make_proj_state()
        for hp in range(N_MT):
            for qm in range(N_QM):
                last = hp == N_MT - 1 and qm == N_QM - 1
                if last and pending is not None:
                    # The deferral exists so epilogue_b's DVE ops can't
                    # head-of-line-block the next period's proj bias-adds;
                    # the proj queue is exhausted before the last period,
                    # so emit period 14's epilogue_b ahead of it and keep
                    # the post-last-exp tail to period 15's epilogue only.
                    epilogue_b(*pending)
                    pending = None
                ctx_ps = new_ctx_ps(hp, qm)
                for kc in range(N_KC):
                    attn_kc(hp, qm, kc, ctx_ps, pstate)
                ep_idx = hp * N_QM + qm
                tfulls = epilogue_a(
                    ctx_ps,
                    use_pe=last,
                    ep_idx=ep_idx,
                )
                if pending is not None:
                    epilogue_b(*pending)
                pending = (hp, qm, tfulls)
        epilogue_b(*pending)


_NC_CACHE = None


def _get_nc():
    global _NC_CACHE
    if _NC_CACHE is None:
        _NC_CACHE = build()
    return _NC_CACHE


def make_in_maps(hidden_states, Wq, bq, Wk, bk, Wv, bv):
    hs = np.ascontiguousarray(np.asarray(hidden_states, dtype=np.float32))
    ws = {k: np.asarray(v, dtype=np.float32)
          for k, v in (("q", Wq), ("k", Wk), ("v", Wv))}
    bs = {k: np.asarray(v, dtype=np.float32)
          for k, v in (("q", bq), ("k", bk), ("v", bv))}
    in_maps = []
    for c in range(NCORES):
        b, g = c // 2, c % 2
        sl = slice(g * HG, (g + 1) * HG)
        in_maps.append({
            "x": np.ascontiguousarray(hs[b]),
            "wq": np.ascontiguousarray(ws["q"][:, sl]),
            "wk": np.ascontiguousarray(ws["k"][:, sl]),
            "wv": np.ascontiguousarray(ws["v"][:, sl]),
            "bq": np.ascontiguousarray(bs["q"][sl]),
            "bk": np.ascontiguousarray(bs["k"][sl]),
            "bv": np.ascontiguousarray(bs["v"][sl]),
        })
    return in_maps


def run(in_maps, trace=False):
    _ensure_profile_hook()
    nc = _get_nc()
    return run_bass_kernel_spmd(nc, in_maps, list(range(NCORES)), trace=trace)


def kernel(hidden_states, Wq, bq, Wk, bk, Wv, bv):
    in_maps = make_in_maps(hidden_states, Wq, bq, Wk, bk, Wv, bv)
    res = run(in_maps, trace=False)
    out = np.empty((B, S, H), dtype=np.float32)
    for c in range(NCORES):
        b, g = c // 2, c % 2
        out[b, :, g * HG:(g + 1) * HG] = res.results[c]["out"]
    return out



# revision 34
# speedup vs baseline: 1.0300x; 1.0300x over previous
"""BERT self-attention (B=4, S=2048, H=1024, 16 heads x 64) on 8 TRN2 NeuronCores.

Sharding: data-parallel over batch (4) x tensor-parallel over head-groups (2).
Core c handles batch c//2 and heads [8*(c%2), 8*(c%2)+8): it gets the full
hidden_states[b] plus the 512 W-columns/bias entries for its heads, and
produces out[b, :, 512*g : 512*(g+1)]. No cross-core communication.

Per-core kernel (bf16/fp16 matmuls, f32 accumulation in PSUM):
  xT   = transpose(x) via PE (bf16)               [1024h, 2048s]
  QT/KT = W.T @ xT  (+bias)                       [512hd, 2048s]
  V'   = xT.T @ Wv (+bias), 65 cols per head with an appended ones column
  per (head-pair, q-macro 512, k-chunk 128):
    scoresT[k, q] = KT_h[:, kc].T @ QT_h[:, qm]   (two heads row-packed, K=64)
    expT = exp(0.125 * scoresT)                   (ACT, fp16 out, N=1024/inst)
    ctxT[65, q] += V'_h[kc].T @ expT              (row 64 = softmax denominator)
    + one Q/K projection matmul of the NEXT head-pair (fills the PE gap
      while ACT paces the loop at ~1.1us/k-chunk)
  epilogue: ctxT -> hardware DMA-transpose -> [q, 65]; multiply by
  reciprocal denominator (GPSIMD); batched DMA out via HWDGE.

Schedule notes (from perfetto traces):
 - ACT exp (FD=1024 from PSUM) is the loop pacer at ~1.07-1.11us/iter;
   the PE instruction stream (ctx pair + row-packed score pair + one
   proj MM) is ~1.08us/iter, so both engines run ~100% in steady state.
   PSUM's 8 banks are exactly spent (2x2 score double-buffer + 2 ctx
   accumulators + 2 proj accumulators), which caps the exp FD at 1024.
 - The epilogue is split: the PSUM-draining copies + DMA-transposes are
   issued at the period boundary, but the reciprocal/scale/store half is
   deferred one full period so its DVE ops sit after the next period's
   proj bias-adds in the DVE FIFO and can never head-of-line-block them
   (that stall idled ACT ~1.7us per boundary and HAM-re-throttled PE).
 - The last ctx of a period can only run after the last exp, which
   starves the PE through the turnover (the 2-slot score ring can't run
   ahead). Chunks kc=1..4's proj+scores are priority-hoisted to the
   period start so the boundary window stays full; the ctx backlog
   drains through the 10-deep exp-tile ring. Hoisting more chunks
   back-fires (priority ties scramble the score-ring order).
 - All staging DMAs go through the sync-engine HWDGE rings (SWDGE
   DIRECT2D enqueues cost ~640ns each on the GPSIMD engine), with x
   prefetched ahead of the wq/wk staging (DMA rings round-robin, so
   enqueue order decides arrival order under bandwidth contention).
 - The Q/K projection interleave is paced globally at 1 matmul per
   iteration across all 16 periods; the queue also carries the qT
   projections of qm1-3 (phase1 only computes kT for windows 1-3 plus
   the qm0 qT), which shaved ~5us off the serial phase1 tail.  Per-iter
   PE (score pair 230ns + ctx pair 430ns + 1 proj MM 215ns + overheads
   ~1.03us) stays just under the exp pace (1.08us); all queue deadlines
   hold with >=4-iteration emission margin (k(3,3) lands at iter 192 vs
   first read at 204).
 - x is DMA'd as 1MB two-tile blocks (each dma_start costs ~0.6-1us of
   Sync-engine enqueue time; halving the enqueue count pulled the x
   arrivals ~4us earlier).
 - The last period's epilogue_b deferral is dropped (the proj queue is
   exhausted by then, so there is nothing to head-of-line-block),
   keeping the post-last-exp tail to ~9us.
Measured on TRN2: ~361.5us HW exec (v1 of this schedule: ~368us;
naive baseline 412us), rel l2 err ~3.7e-3.

Optimization attempts that did NOT beat this schedule (2026-08-12
session; all correct at 3.7e-3 but slower — details in case a future
session retries):
 - Streaming phase1 ("flash-style" period-0 crawl: x DMA'd in windows,
   transposes/V'/mt0-proj emitted per window interleaved with period 0's
   scores/exps; W staged as per-head-pair column slices; dummy exp at
   t=0 to preload the ACT table): 382.7us.  The exp stream starts at
   ~17us instead of ~90us, but the crawl's PE backlog (V' 27.5us +
   transposes + mt0 proj = ~50us of PE that all sits ahead of later
   scores in the PE FIFO) scatters ~60-90us of ACT idle through
   t=17-100us, and the steady state picks up ~0.03-0.1us/iter of ring
   coupling noise.  Engine totals measured: ACT-busy ~285us (256 exps x
   ~1.11us) vs PE-busy ~300us — the PE, not ACT, is the binding total,
   so any schedule floor is ~310us + head/tail; none of the streaming
   variants got close to that floor due to FIFO coupling losses.
 - Same + wavefront (qm1's exps pre-run in the crawl via an 18-deep exp
   ring, ctx replayed later): 407.7us (deferred-ctx ring pins + ACT
   FIFO ordering of PSUM->SBUF transpose-drains vs exps serialize the
   window chain).  With transpose-drains moved DVE->ACT: 471us.
 - v1 + mild overlap (period 0's 16 attn iters emitted into phase1's
   st>=13 tail, queue prefix pre-run to keep the mt3 kT deadline):
   375.7us; with ps_sc/ps_pj ring decoupling + casts off ACT: 378.1us.
   The phase1 st-loop is balanced enough that inserting exps stretches
   it by more than the head saving.  (Confirmed via PE-interval
   analysis: PE is ~92% occupied wall-to-wall — 324.6us busy in a
   353.6us span, only 6us of >1us gaps — so the 16-18us ACT-dead
   transition gap at the phase1/steady boundary is PE catch-up time,
   not reclaimable idle.)
 - matmul cannot write 16-bit PSUM (bass asserts fp32 out), so the
   exp FD cannot be doubled to 2048 by halving the score footprint.
 - Period-0 scores+exps moved into phase1's st12-15 era (ctx trailing
   one window, kproj/transposes on the ps_pj ring, casts on DVE, one
   queue group pre-run for the mt3 deadline): 366.5us vs this
   schedule's 361.5us.  The phase1->steady gap shrank (18.4->16.5us)
   but periods 1-2 picked up ~11us of new stalls — the PE work that
   feeds them did not shrink, it was only displaced later.  This closes
   the book on head-overlap schedules: the transition gap is PE-bound
   catch-up, full stop.
 - PE warm-up burst (100 identity matmuls at t~0 to trip the HAM
   clock gate before the first transposes): 430us — the dummies all
   write one PSUM tile, so they form a WAW chain that the scheduler
   serializes with per-dependency semaphores, stalling the whole PE
   stream.  Splitting the first 1MB x block into two 0.5MB tiles (to
   start the cast chain ~1.8us earlier): 364.7us — the extra enqueue
   and changed arrival pattern cost more than the earlier start.
 - Facts for future attempts: GPSIMD tensor_copy/CAST is ~5 cyc/elem
   ([128,1024] cast = 3.0us, bf16 copy [128,512] = 1.87us) — useless
   for bulk staging; DVE CAST [128,1024] = 0.68us; ACT copy =
   (N+352)/1.2ns; each nc.sync.dma_start costs ~0.6-1.2us of Sync-
   engine enqueue time and a [0,P]-broadcast DMA costs ~4.3us to
   enqueue; the runtime preamble before the first DMA can issue is
   ~7us; exp ACTIVATE measures ~1.11us (FD=1024 from PSUM); the
   row-packed K=64 score pair runs concurrently (~230ns/pair); ctx
   pair is serial (~430ns); per-iter steady PE ~0.86us + 7/8-paced
   proj ~0.19us, just under the exp pace.
"""

import sys
import types

sys.path.insert(0, "/opt/trn_rl_repo")

import numpy as np

import concourse.bass as bass
import concourse.tile as tile
from concourse import bacc, mybir
from concourse.bass_utils import run_bass_kernel_spmd
from concourse.masks import make_identity

B, S, H = 4, 2048, 1024
NH, HD = 16, 64
NCORES = 8
HEADS_PER_CORE = NH // 2      # 8 heads per core
HG = HEADS_PER_CORE * HD      # 512 = per-core head width
P = 128
QM = 512                      # q macro-tile
N_QM = S // QM                # 4
N_KC = S // P                 # 16 k chunks
N_ST = S // P                 # 16 s tiles
N_HB = H // P                 # 8 h chunks (contraction)
N_MT = HG // P                # 4 hd m-tiles

FP32 = mybir.dt.float32
BF16 = mybir.dt.bfloat16
FP16 = mybir.dt.float16


def _ensure_profile_hook():
    """The image's antenv lacks axon_hooks; shim it so trace=True works."""
    try:
        from antenv.axon_hooks import get_axon_ntff_profile_hook  # noqa: F401
        return
    except ImportError:
        pass
    try:
        from trn_agent_boot.trn_boot import _ntff_profile_via_ctypes
    except ImportError:
        return
    hook = _ntff_profile_via_ctypes("/opt/axon/libaxon_pjrt.so")
    mod = types.ModuleType("antenv.axon_hooks")
    mod.get_axon_ntff_profile_hook = lambda: hook
    mod.set_axon_ntff_profile_hook = lambda h: None
    sys.modules["antenv.axon_hooks"] = mod


def build():
    nc = bacc.Bacc("TRN2", target_bir_lowering=False, debug=False,
                   num_devices=NCORES)

    x_d = nc.declare_dram_parameter("x", [S, H], FP32, isOutput=False)
    wq_d = nc.declare_dram_parameter("wq", [H, HG], FP32, isOutput=False)
    wk_d = nc.declare_dram_parameter("wk", [H, HG], FP32, isOutput=False)
    wv_d = nc.declare_dram_parameter("wv", [H, HG], FP32, isOutput=False)
    bq_d = nc.declare_dram_parameter("bq", [HG], FP32, isOutput=False)
    bk_d = nc.declare_dram_parameter("bk", [HG], FP32, isOutput=False)
    bv_d = nc.declare_dram_parameter("bv", [HG], FP32, isOutput=False)
    out_d = nc.declare_dram_parameter("out", [S, HG], FP32, isOutput=True)

    with tile.TileContext(nc) as tc:
        _build_body(nc, tc, x_d, (wq_d, wk_d, wv_d), (bq_d, bk_d, bv_d), out_d)

    nc.finalize()
    return nc


def _build_body(nc, tc, x_d, w_d, b_d, out_d):
    wq_d, wk_d, wv_d = w_d
    bq_d, bk_d, bv_d = b_d

    import contextlib
    from contextlib import nullcontext as _nullcontext
    ctx = contextlib.ExitStack()
    with ctx:
        const = ctx.enter_context(tc.tile_pool(name="const", bufs=1))
        xf = ctx.enter_context(tc.tile_pool(name="xf", bufs=5))
        xbp = ctx.enter_context(tc.tile_pool(name="xbp", bufs=3))
        big = ctx.enter_context(tc.tile_pool(name="big", bufs=1))
        wstage = ctx.enter_context(tc.tile_pool(name="wstage", bufs=8))
        expp = ctx.enter_context(tc.tile_pool(name="expp", bufs=10))
        epil = ctx.enter_context(tc.tile_pool(name="epil", bufs=4))
        outp = ctx.enter_context(tc.tile_pool(name="outp", bufs=4))
        # PSUM budget (8 banks): ps_sc = 2 x 2-bank slots (scores double
        # buffer), ps_ctx = 2 x 1-bank slots (ctx accumulators), ps_pj =
        # 2 x 1-bank slots (V'/QK projection accumulators).
        ps_sc = ctx.enter_context(
            tc.tile_pool(name="ps_sc", bufs=2, space="PSUM"))
        ps_ctx = ctx.enter_context(
            tc.tile_pool(name="ps_ctx", bufs=2, space="PSUM"))
        ps_pj = ctx.enter_context(
            tc.tile_pool(name="ps_pj", bufs=2, space="PSUM"))

        # ---- x prefetch first so the first s-tiles land ASAP -----------
        x_blks = {}
        x_tiles = {}

        def fetch_x(st):
            # 1MB two-tile blocks: each dma_start costs ~0.6-1us of Sync
            # engine enqueue time, so halve the enqueue count.
            w = st // 2
            if w in x_blks:
                return
            xt = xf.tile([P, 2, H], FP32, tag="x", name=f"xblk{w}")
            nc.sync.dma_start(
                out=xt,
                in_=x_d.ap()[2 * w * P:2 * (w + 1) * P, :].rearrange(
                    "(a p) c -> p a c", p=P),
            )
            x_blks[w] = xt
            x_tiles[2 * w] = xt[:, 0, :]
            x_tiles[2 * w + 1] = xt[:, 1, :]

        for st in range(4):
            fetch_x(st)

        # ---- constants -------------------------------------------------
        ident_b128 = const.tile([P, P], BF16)
        make_identity(nc, ident_b128)
        ident_h = const.tile([HD + 1, HD + 1], FP16)
        make_identity(nc, ident_h)
        ident_b = ident_h
        bqT = const.tile([P, N_MT], FP32)
        nc.sync.dma_start(out=bqT, in_=bq_d.ap().rearrange("(o p) -> p o", p=P))
        bkT = const.tile([P, N_MT], FP32)
        nc.sync.dma_start(out=bkT, in_=bk_d.ap().rearrange("(o p) -> p o", p=P))
        bv_ap = bv_d.ap()
        bvb = const.tile([P, HG], FP32)
        nc.sync.dma_start(
            out=bvb,
            in_=bass.AP(tensor=bv_ap.tensor, offset=bv_ap.offset,
                        ap=[[0, P]] + [list(a) for a in bv_ap.ap]),
        )

        # ---- weights: staged via sync HWDGE, cast on DVE ---------------
        w_sb = {}
        for name, wd in (("q", wq_d), ("k", wk_d), ("v", wv_d)):
            w_sb[name] = big.tile([P, N_HB, HG], BF16, tag=f"w{name}",
                                  name=f"w{name}")

        wpre = {}

        def load_w(name, wd, ks):
            for k in ks:
                stg = wstage.tile([P, HG], FP32, tag="wstg", name=f"w{name}{k}")
                nc.sync.dma_start(out=stg, in_=wd.ap()[k * P:(k + 1) * P, :])
                nc.vector.tensor_copy(out=w_sb[name][:, k, :], in_=stg)

        def dma_w(name, wd, k):
            stg = wstage.tile([P, HG], FP32, tag="wstg", name=f"w{name}{k}")
            nc.sync.dma_start(out=stg, in_=wd.ap()[k * P:(k + 1) * P, :])
            wpre[(name, k)] = stg

        def cast_w(name, ks):
            for k in ks:
                nc.vector.tensor_copy(out=w_sb[name][:, k, :],
                                      in_=wpre.pop((name, k)))

        load_w("v", wv_d, range(N_HB))
        # All remaining input DMAs enqueued NOW in deadline order (the
        # HWDGE ring is FIFO): x blocks early, wq before wk.  The st loop
        # keeps only the casts — v1's in-loop stagger enqueued ~1.5MB/st
        # against ~1.15MB/st of ring throughput, so x arrivals ran a
        # growing deficit that showed up as 3-8us ACT gaps mid-phase1
        # and run-to-run jitter under HBM contention.
        fetch_x(4)
        for k in range(N_HB):
            dma_w("q", wq_d, k)
        fetch_x(6)
        fetch_x(8)
        fetch_x(10)
        for k in range(N_HB):
            dma_w("k", wk_d, k)
        fetch_x(12)
        fetch_x(14)

        # ---- per s-tile: load x, transpose to xT, project V' -----------
        xT = big.tile([P, N_HB, S], BF16, tag="xT")
        vp = big.tile([P, N_ST, HEADS_PER_CORE, HD + 1], BF16, tag="vp")
        # only the appended ones-column needs initializing; cols 0:HD are
        # fully written by the V' bias add below
        nc.vector.memset(vp[:, :, :, HD:HD + 1], 1.0)

        qT = big.tile([P, N_MT, S], BF16, tag="qT")
        kT = big.tile([P, N_MT, S], BF16, tag="kT")

        def proj_chunk(mt, n, pool=None, tag=None, names=("q", "k")):
            pool = pool or ps_pj
            tag = tag or "pj"
            for w_name, dst, bias in (("q", qT, bqT), ("k", kT, bkT)):
                if w_name not in names:
                    continue
                ps = pool.tile([P, QM], FP32, tag=tag,
                               name=f"proj{w_name}{mt}{n}")
                for k in range(N_HB):
                    nc.tensor.matmul(
                        ps,
                        lhsT=w_sb[w_name][:, k, mt * P:(mt + 1) * P],
                        rhs=xT[:, k, n * QM:(n + 1) * QM],
                        start=(k == 0),
                        stop=(k == N_HB - 1),
                    )
                nc.vector.tensor_scalar_add(
                    out=dst[:, mt, n * QM:(n + 1) * QM],
                    in0=ps,
                    scalar1=bias[:, mt:mt + 1],
                )

        def vprime(st):
            psv = ps_pj.tile([P, HG], FP32, tag="pj", name=f"v{st}")
            for hb in range(N_HB):
                nc.tensor.matmul(
                    psv,
                    lhsT=xT[:, hb, st * P:(st + 1) * P],
                    rhs=w_sb["v"][:, hb, :],
                    start=(hb == 0),
                    stop=(hb == N_HB - 1),
                )
            nc.vector.scalar_tensor_tensor(
                out=vp[:, st, :, 0:HD],
                in0=psv.rearrange("p (h d) -> p h d", h=HEADS_PER_CORE),
                scalar=1.0,
                in1=bvb.rearrange("p (h d) -> p h d", h=HEADS_PER_CORE),
                op0=mybir.AluOpType.mult,
                op1=mybir.AluOpType.add,
            )

        def phase1(ctx_ps0, pj0):
          # V' for tile st-1 is emitted after the transposes of tile st so
          # the PE never waits on the PSUM->SBUF eviction of its own tile.
          for st in range(N_ST):
            if 3 <= st < 7:
                cast_w("q", (2 * (st - 3), 2 * (st - 3) + 1))
            elif 7 <= st < 11:
                cast_w("k", (2 * (st - 7), 2 * (st - 7) + 1))
            xt = x_tiles[st]
            xb = xbp.tile([P, H], BF16, tag="xb", name=f"xb{st}")
            nc.scalar.copy(out=xb, in_=xt)
            for half in range(2):
                ps = ps_sc.tile([P, 4, P], BF16, tag="sc", name=f"xt{st}{half}")
                for q in range(4):
                    hb = half * 4 + q
                    nc.tensor.transpose(
                        ps[:, q, :], xb[:, hb * P:(hb + 1) * P], ident_b128)
                nc.vector.tensor_copy(
                    out=xT[:, half * 4:half * 4 + 4, st * P:(st + 1) * P],
                    in_=ps,
                )
            if st > 0:
                vprime(st - 1)
            if st >= 12:
                # kT windows are consumed inside period 0 itself; the qT
                # of qm1-3 ride the (now 8/8-paced) steady queue instead,
                # shaving ~5us of PE off the serial phase1 tail.
                proj_chunk(0, st - 12, pool=ps_ctx, tag="ctx",
                           names=("q", "k") if st == 12 else ("k",))
          vprime(N_ST - 1)

        # ---- attention for one head pair -------------------------------
        PD = 80  # 65 padded to a multiple of XBAR_TILE_SRC_ROWS (16)

        def new_ctx_ps(hp, qm):
            return [ps_ctx.tile([HD + 1, QM], FP32, tag="ctx",
                                name=f"ctx{hp}{qm}{hh}")
                    for hh in range(2)]

        # The Q/K projections for later head-pairs are paced globally at
        # ~7 matmuls per 8 iterations across ALL periods (instead of 1 per
        # iteration during hp0-2 and none during hp3): the exp instruction
        # only runs at its pure ~1005ns when the PE keeps the score feed a
        # full iteration ahead, and that needs per-iteration PE work at or
        # below the exp duration. Chunk deadlines (hp_next's qT/kT before
        # hp_next's periods) are met with large margin at this pacing.
        def make_proj_state():
            queue = [("q", 0, n) for n in (1, 2, 3)]
            for nxt in (1, 2, 3):
                queue.append(("q", nxt, 0))
                queue.extend(("k", nxt, n) for n in range(N_QM))
                queue.extend(("q", nxt, n) for n in (1, 2, 3))
            return {"queue": queue, "ci": 0, "mi": 0, "tile": None}

        def attn_kc(hp, qm, kc, ctx_ps, pstate):
            # The last ctx of a period can only run after the last exp, so
            # the PE would starve through the period turnover. Hoisting the
            # first few chunks' proj+scores to the period start keeps the
            # PE (and therefore the ACT's score feed) saturated while the
            # ctx backlog drains through the exp-tile ring.
            hoist = (tc.high_priority(offset=6 * kc) if kc in (1, 2, 3, 4)
                     else _nullcontext())
            with hoist:
                if pstate["ci"] < len(pstate["queue"]):
                    w_name, nxt, n = pstate["queue"][pstate["ci"]]
                    mi = pstate["mi"]
                    if mi == 0:
                        pstate["tile"] = ps_pj.tile(
                            [P, QM], FP32, tag="pj",
                            name=f"pj{w_name}{nxt}{n}")
                    nc.tensor.matmul(
                        pstate["tile"],
                        lhsT=w_sb[w_name][:, mi, nxt * P:(nxt + 1) * P],
                        rhs=xT[:, mi, n * QM:(n + 1) * QM],
                        start=(mi == 0),
                        stop=(mi == N_HB - 1),
                    )
                    if mi == N_HB - 1:
                        dst, bias = (qT, bqT) if w_name == "q" else (kT, bkT)
                        nc.vector.tensor_scalar_add(
                            out=dst[:, nxt, n * QM:(n + 1) * QM],
                            in0=pstate["tile"],
                            scalar1=bias[:, nxt:nxt + 1],
                        )
                        pstate["ci"] += 1
                        pstate["mi"] = 0
                    else:
                        pstate["mi"] = mi + 1
                sc = ps_sc.tile([P, 2, QM], FP32, tag="sc",
                                name=f"sc{hp}{qm}{kc}")
                for hh in range(2):
                    lo = hh * HD
                    nc.tensor.matmul(
                        sc[:, hh, :],
                        lhsT=kT[lo:lo + HD, hp, kc * P:(kc + 1) * P],
                        rhs=qT[lo:lo + HD, hp, qm * QM:(qm + 1) * QM],
                        start=True,
                        stop=True,
                        tile_position=(lo, 0),
                    )
            et = expp.tile([P, 2, QM], FP16, tag="exp")
            nc.scalar.activation(
                out=et, in_=sc,
                func=mybir.ActivationFunctionType.Exp,
                scale=0.125,
            )
            for hh in range(2):
                nc.tensor.matmul(
                    ctx_ps[hh],
                    lhsT=vp[:, kc, 2 * hp + hh, :],
                    rhs=et[:, hh, :],
                    start=(kc == 0),
                    stop=(kc == N_KC - 1),
                )

        def epilogue_a(ctx_ps, use_pe, ep_idx):
            """Drain the ctx accumulators out of PSUM and kick off the
            transposes. Emitted at the period boundary so the PSUM slots
            free quickly for the next period's accumulation."""
            tfulls = []
            for hh in range(2):
                csb = epil.tile([PD, QM], FP16, tag="ctxsb")
                if ep_idx < 2:
                    # rows 65:PD feed the xbar transpose as padding; each
                    # of the 4 ring buffers only needs zeroing once.
                    nc.vector.memset(csb[64:PD, :], 0.0)
                nc.vector.tensor_copy(out=csb[0:HD + 1, :], in_=ctx_ps[hh])
                if use_pe:
                    tp = ps_pj.tile([P, QM // P, HD + 2], FP16, tag="pj",
                                    name=f"tp{hh}")
                    for qs in range(QM // P):
                        nc.tensor.transpose(
                            tp[:, qs, 0:HD + 1],
                            csb[0:HD + 1, qs * P:(qs + 1) * P],
                            ident_b,
                        )
                    tfull = epil.tile([P, QM // P, HD + 1], FP16, tag="tpe")
                    nc.vector.tensor_copy(out=tfull, in_=tp[:, :, 0:HD + 1])
                else:
                    tfull = epil.tile([P, QM // P, PD], FP16, tag="tpsb")
                    for qs in range(QM // P):
                        nc.sync.dma_start_transpose(
                            out=tfull[:, qs, :],
                            in_=csb[:, qs * P:(qs + 1) * P],
                        )
                tfulls.append(tfull)
            return tfulls

        def epilogue_b(hp, qm, tfulls):
            """Reciprocal + scale + store. Deferred one period so these DVE
            ops are emitted after the next period's proj bias-adds and can
            never head-of-line-block them (the transposes they read were
            issued a full period earlier and are long done)."""
            for hh in range(2):
                tfull = tfulls[hh]
                rc = outp.tile([P, QM // P], FP32, tag="recip")
                nc.vector.reciprocal(out=rc, in_=tfull[:, :, HD:HD + 1])
                ot = outp.tile([P, QM // P, HD], FP32, tag="out")
                for qs in range(QM // P):
                    nc.vector.tensor_scalar_mul(
                        ot[:, qs, :], tfull[:, qs, 0:HD], rc[:, qs:qs + 1])
                row = qm * QM
                col = (2 * hp + hh) * HD
                nc.sync.dma_start(
                    out=out_d.ap()[row:row + QM, col:col + HD].rearrange(
                        "(a p) c -> p a c", p=P),
                    in_=ot,
                )

        phase1(None, None)
        pending = None
        pstate = make_proj_state()
        for hp in range(N_MT):
            for qm in range(N_QM):
                last = hp == N_MT - 1 and qm == N_QM - 1
                if last and pending is not None:
                    # The deferral exists so epilogue_b's DVE ops can't
                    # head-of-line-block the next period's proj bias-adds;
                    # the proj queue is exhausted before the last period,
                    # so emit period 14's epilogue_b ahead of it and keep
                    # the post-last-exp tail to period 15's epilogue only.
                    epilogue_b(*pending)
                    pending = None
                ctx_ps = new_ctx_ps(hp, qm)
                for kc in range(N_KC):
                    attn_kc(hp, qm, kc, ctx_ps, pstate)
                ep_idx = hp * N_QM + qm
                tfulls = epilogue_a(
                    ctx_ps,
                    use_pe=last,
                    ep_idx=ep_idx,
                )
                if pending is not None:
                    epilogue_b(*pending)
                pending = (hp, qm, tfulls)
        epilogue_b(*pending)


_NC_CACHE = None


def _get_nc():
    global _NC_CACHE
    if _NC_CACHE is None:
        _NC_CACHE = build()
    return _NC_CACHE


def make_in_maps(hidden_states, Wq, bq, Wk, bk, Wv, bv):
    hs = np.ascontiguousarray(np.asarray(hidden_states, dtype=np.float32))
    ws = {k: np.asarray(v, dtype=np.float32)
          for k, v in (("q", Wq), ("k", Wk), ("v", Wv))}
    bs = {k: np.asarray(v, dtype=np.float32)
          for k, v in (("q", bq), ("k", bk), ("v", bv))}
    in_maps = []
    for c in range(NCORES):
        b, g = c // 2, c % 2
        sl = slice(g * HG, (g + 1) * HG)
        in_maps.append({
            "x": np.ascontiguousarray(hs[b]),
            "wq": np.ascontiguousarray(ws["q"][:, sl]),
            "wk": np.ascontiguousarray(ws["k"][:, sl]),
            "wv": np.ascontiguousarray(ws["v"][:, sl]),
            "bq": np.ascontiguousarray(bs["q"][sl]),
            "bk": np.ascontiguousarray(bs["k"][sl]),
            "bv": np.ascontiguousarray(bs["v"][sl]),
        })
    return in_maps


def run(in_maps, trace=False):
    _ensure_profile_hook()
    nc = _get_nc()
    return run_bass_kernel_spmd(nc, in_maps, list(range(NCORES)), trace=trace)


def kernel(hidden_states, Wq, bq, Wk, bk, Wv, bv):
    in_maps = make_in_maps(hidden_states, Wq, bq, Wk, bk, Wv, bv)
    res = run(in_maps, trace=False)
    out = np.empty((B, S, H), dtype=np.float32)
    for c in range(NCORES):
        b, g = c // 2, c % 2
        out[b, :, g * HG:(g + 1) * HG] = res.results[c]["out"]
    return out

